# revision 12
# baseline (speedup 1.0000x reference)
"""GAT (2-layer, PyG-style) on 8 Trainium2 NeuronCores.

Strategy (edge-parallel, dst-sharded), v4:
  - Host sorts non-self edges by dst, assigns dst-ranges of 6250 nodes to
    each of 8 cores, tiles each core's nodes into 128-node groups, and
    chunks each group's edges into 128-edge chunks (split by src<32768
    parity because dma_gather indices are int16).  Self-loops are NOT in
    the edge stream: each group gets one synthetic "self chunk" whose
    sources are the group's own nodes (no gather needed).
  - Layer 1 never materializes an h table: each chunk GATHERS raw x rows
    (256B, TRANSPOSED so features land on partitions, matmul-lhsT-ready)
    and recomputes h = x@W1 and als = x@A_s per chunk on the PE.  The dst
    term ald is expanded per edge by a matmul against the transposed
    selection matrix ST and accumulated INTO THE SAME PSUM bank as als,
    so e = als+ald needs no vector work.  exp(leaky_relu(e)) =
    max(exp(s*e), exp(e)) on the scalar engine straight out of PSUM.
    Messages mt = h * ex (vector), then selection-matrix matmuls
    accumulate num/den per 128-node dst group in PSUM.  No AllGather for
    layer 1 at all; gathers start at t=0 and pipeline across all 4 SWDGE
    queues (Q7 desc-gen offloads per queue; deep buffering keeps several
    transfers in flight).
  - Layer 2 keeps a small AllGathered g2 table ([1 | g2 | als2] rows so
    the softmax denominator rides along the message matmul for free); its
    dst column ald2 stays in SBUF from phase C and is expanded with the
    same ST trick.  Self chunks read a resident SBUF copy of own g2 rows.
  - Mean-pool partials via one selection matmul + AllReduce; fc +
    log_softmax replicated on every core.
"""

import os
import sys

sys.path.insert(0, "/opt/trn_rl_repo")

import numpy as np
import ml_dtypes

BF16 = ml_dtypes.bfloat16

# problem constants (hardcoded per contract)
N = 50000
E0 = 400000
F = 128
HID = 64
H1 = 8
HC = 512  # H1*HID
G = 64
CLS = 10
SLOPE = 0.2
NCORES = 8
NPC = N // NCORES  # 6250
NT = (NPC + 127) // 128  # 49
NPAD = NT * 128  # 6272
SPLIT = 32768
CB = int(os.environ.get("GAT_CB", "16"))  # chunks per gather batch
EB = CB  # chunks per psumE bank (one bank's group == one d-batch)
NQ = int(os.environ.get("GAT_NQ", "4"))  # SWDGE queues gathers rotate across
GBUFS = int(os.environ.get("GAT_GBUFS", "8"))  # gather buffers in flight
G2W = HID + 2  # g2 table row: [1 | g2(HID) | als2]


def _set_size(n, e0, split, cb):
    """Debug helper: shrink the problem for simulator runs."""
    global N, E0, NPC, NT, NPAD, SPLIT, CB, EB
    N, E0, SPLIT, CB = n, e0, split, cb
    NPC = N // NCORES
    NT = (NPC + 127) // 128
    NPAD = NT * 128
    EB = CB


def _wrap_idx(idx):
    """[M] int -> [128, M//16] int16 in the dma_gather wrapped layout."""
    M = len(idx)
    assert M % 16 == 0
    a = np.asarray(idx, dtype=np.int16).reshape(M // 16, 16).T  # [16, M/16]
    return np.tile(a, (8, 1)).copy()  # [128, M/16]


def preprocess(edge_index, batch):
    """Build the shared chunk schedule plus per-core index/side arrays."""
    # NOTE: accidental self-edges in edge_index stay in the stream; only the
    # reference's synthetic arange() loops become per-group self chunks.
    src = np.asarray(edge_index[0], np.int64)
    dst = np.asarray(edge_index[1], np.int64)
    order = np.argsort(dst, kind="stable")
    src, dst = src[order], dst[order]

    # bucket[core][group][parity] -> (src_list, dst_list)
    buckets = [[[None, None] for _ in range(NT)] for _ in range(NCORES)]
    core_of = dst // NPC
    for k in range(NCORES):
        m = core_of == k
        s_k, d_k = src[m], dst[m]
        dloc = d_k - NPC * k
        g_k = dloc // 128
        p_k = (s_k >= SPLIT).astype(np.int64)
        keys = g_k * 2 + p_k
        o2 = np.argsort(keys, kind="stable")
        s_k, d_k, keys = s_k[o2], d_k[o2], keys[o2]
        bounds = np.searchsorted(keys, np.arange(2 * NT + 1))
        for g in range(NT):
            for p in range(2):
                lo, hi = bounds[2 * g + p], bounds[2 * g + p + 1]
                buckets[k][g][p] = (s_k[lo:hi], d_k[lo:hi])

    # shared chunk counts (regular chunks; self chunk is extra, 1 per group)
    nch = np.zeros((NT, 2), dtype=np.int64)
    for g in range(NT):
        for p in range(2):
            mx = max(len(buckets[k][g][p][0]) for k in range(NCORES))
            nch[g, p] = (mx + 127) // 128

    # shared schedule: per group, [self chunk] then regular chunks per parity
    chunks = []  # dicts: g, p, sslot(None for self), c, first, last
    scount = [0, 0]
    for g in range(NT):
        chunks.append(dict(g=g, p=None, sslot=None, c=len(chunks), first=True, last=False))
        for p in range(2):
            for _ in range(nch[g, p]):
                chunks.append(
                    dict(g=g, p=p, sslot=scount[p], c=len(chunks), first=False, last=False)
                )
                scount[p] += 1
        chunks[-1]["last"] = True
    NCH = len(chunks)
    NSL, NSH = scount
    NBL = (NSL + CB - 1) // CB
    NBH = (NSH + CB - 1) // CB
    NBD = (NCH + CB - 1) // CB

    # runs over REGULAR chunks: maximal consecutive chunk spans, same parity,
    # same group, not crossing CB (d-batch or src-batch) boundaries
    runs = []  # (c0, r, p, s0)
    i = 0
    while i < NCH:
        c0 = chunks[i]
        if c0["sslot"] is None:
            i += 1
            continue
        j = i + 1
        while (
            j < NCH
            and chunks[j]["sslot"] is not None
            and chunks[j]["p"] == c0["p"]
            and chunks[j]["g"] == c0["g"]
            and chunks[j]["c"] // CB == c0["c"] // CB
            and chunks[j]["sslot"] // CB == c0["sslot"] // CB
            and chunks[j]["sslot"] == c0["sslot"] + (j - i)
        ):
            j += 1
        runs.append((c0["c"], j - i, c0["p"], c0["sslot"]))
        i = j

    # per-core arrays
    per_core = []
    for k in range(NCORES):
        sidx = [np.zeros(NBL * CB * 128, np.int64) - 1, np.zeros(NBH * CB * 128, np.int64) - 1]
        dstlocT = np.full((128, NBD * CB), -1.0, np.float32)
        for ch in chunks:
            g, p, ss, c = ch["g"], ch["p"], ch["sslot"], ch["c"]
            if ss is None:
                # self chunk: dstloc = iota over the group's valid nodes
                nt_ = min(128, NPC - 128 * g)
                dl = np.full(128, -1.0, np.float32)
                dl[:nt_] = np.arange(nt_, dtype=np.float32)
                dstlocT[:, c] = dl
                continue
            s_e, d_e = buckets[k][g][p]
            ne = len(s_e)
            sv = np.zeros(128, np.int64)
            dl = np.full(128, -1.0, np.float32)
            jprev = ss - sum(nch[gg, p] for gg in range(g))
            lo = jprev * 128
            hi = min(lo + 128, ne)
            nval = max(0, hi - lo)
            if nval > 0:
                sv[:nval] = s_e[lo:hi]
                dl[:nval] = (d_e[lo:hi] - (NPC * k + 128 * g)).astype(np.float32)
            if p == 1:
                sv = np.where(sv >= SPLIT, sv - SPLIT, 0)
            sidx[p][ss * 128 : ss * 128 + 128] = sv
            dstlocT[:, c] = dl
        # precomputed one-hot pooling selection: gtT[p, t*G+g] = (graph of
        # node 128t+p == g); pad rows stay 0
        gtT = np.zeros((128, NT * G), np.float32)
        for t in range(NT):
            n0 = NPC * k + 128 * t
            nt_ = min(128, NPC * (k + 1) - n0)
            gids = batch[n0 : n0 + nt_].astype(np.int64)
            gtT[np.arange(nt_), t * G + gids] = 1.0
        per_core.append(
            dict(
                sidx_lo=_wrap_idx(sidx[0]),
                sidx_hi=_wrap_idx(sidx[1]),
                dstlocT=dstlocT,
                gtT=gtT.astype(BF16),
            )
        )

    sched = dict(chunks=chunks, runs=runs, NCH=NCH, NSL=NSL, NSH=NSH, NBL=NBL, NBH=NBH, NBD=NBD)
    return sched, per_core


def build_program(sched):
    """Build the (shared) 8-core bass program for the given schedule."""
    import concourse.bass as bass
    import concourse.tile as tile
    from concourse import bacc, mybir

    f32 = mybir.dt.float32
    bf16 = mybir.dt.bfloat16
    i16 = mybir.dt.int16
    AF = mybir.ActivationFunctionType
    OP = mybir.AluOpType

    NCH, NBL, NBH, NBD = sched["NCH"], sched["NBL"], sched["NBH"], sched["NBD"]
    chunks, runs = sched["chunks"], sched["runs"]

    nc = bacc.Bacc(
        "TRN2",
        target_bir_lowering=False,
        debug=False,
        enable_asserts=False,
        num_swdge_queues=NQ,
        num_devices=NCORES,
    )

    # ---- I/O ----
    def din(name, shape, dt):
        return nc.dram_tensor(name, shape, dt, kind="ExternalInput")

    xb_full = din("xb_full", [N, F], bf16)  # gather table (replicated)
    xTown = din("xTown", [F, NPC], f32)
    w1b = din("w1b", [F, HC], bf16)
    asb = din("asb", [F, H1], bf16)
    adf = din("adf", [F, H1], f32)
    w2e = din("w2e", [HC, HID + 2], bf16)
    fcwb = din("fcwb", [HID + 1, CLS], f32)
    sidx_lo = din("sidx_lo", [128, NBL * CB * 8], i16)
    sidx_hi = din("sidx_hi", [128, NBH * CB * 8], i16)
    # per d-batch: CB dstloc columns (for S) + CB*128 replicated rows (for ST)
    DCW = CB + CB * 128
    dcomb = din("dcomb", [128, NBD * DCW], bf16)
    gtT = din("gtT", [128, NT * G], bf16)
    out = nc.dram_tensor("out", [G, CLS], f32, kind="ExternalOutput")

    iota_np = np.tile(np.arange(128, dtype=np.float32), (128, 1))
    iotab_dram = nc.inline_tensor(iota_np.astype(BF16), name="iota128b")
    iotap_np = np.arange(128, dtype=np.float32).reshape(128, 1).astype(BF16)
    iotap_dram = nc.inline_tensor(iotap_np, name="iotaP128")

    # ---- internal DRAM ----
    h2_kind = (
        "ExternalOutput" if os.environ.get("GAT_DEBUG_H2", "0") == "1" else "Internal"
    )
    h2_dram = nc.dram_tensor("h2_dram", [NPAD, HC], bf16, kind=h2_kind)
    # staging ring for layer-1 gathered x batches (gather -> DRAM -> transposed
    # load, since the SWDGE transpose-gather's completion signal races on HW)
    xstage = nc.dram_tensor("xstage", [GBUFS, CB * 128, F], bf16)
    g2_own = nc.dram_tensor("g2_own", [NPC, 128], bf16)
    g2_full = nc.dram_tensor("g2_full", [N, 128], bf16, addr_space="Shared")
    pool_own = nc.dram_tensor("pool_own", [HID + 1, G], f32)
    pool_ar = nc.dram_tensor("pool_ar", [HID + 1, G], f32, addr_space="Shared")
    pool_loc = nc.dram_tensor("pool_loc", [HID + 1, G], f32)

    RG = [list(range(NCORES))]

    with tile.TileContext(nc) as tc:
        with tc.tile_pool(name="const", bufs=1) as cpool:
            iotab_sb = cpool.tile([128, 128], bf16)
            nc.sync.dma_start(iotab_sb[:], iotab_dram[:])
            iotap_sb = cpool.tile([128, 1], bf16)
            nc.sync.dma_start(iotap_sb[:], iotap_dram[:])
            w1b_sb = cpool.tile([F, HC], bf16)
            nc.sync.dma_start(w1b_sb[:], w1b[:])
            asb_sb = cpool.tile([F, H1], bf16)
            nc.sync.dma_start(asb_sb[:], asb[:])
            adf_sb = cpool.tile([F, H1], f32)
            nc.sync.dma_start(adf_sb[:], adf[:])
            gt_sb = cpool.tile([128, NT * G], bf16)
            nc.sync.dma_start(gt_sb[:], gtT[:])
            # preloaded gather indices (shared by phases B and D)
            sxl_sb = cpool.tile([128, NBL * CB * 8], i16)
            nc.sync.dma_start(sxl_sb[:], sidx_lo[:])
            sxh_sb = cpool.tile([128, NBH * CB * 8], i16)
            nc.sync.dma_start(sxh_sb[:], sidx_hi[:])
            # resident own-x (bf16, transposed) for self chunks
            xbown_sb = cpool.tile([F, NPAD], bf16)
            # per-group dst attention tables, filled by phases A and C
            aldg_sb = cpool.tile([128, NT * H1], bf16)
            ald2g_sb = cpool.tile([128, NT], bf16)
            # resident own-g2 rows for phase D self chunks
            g2own_sb = cpool.tile([128, NT * G2W], bf16)
            ones_col = cpool.tile([128, 1], bf16)
            nc.gpsimd.memset(ones_col[:], 1.0)

            PHASES = os.environ.get("GAT_PHASES", "ABCDE")
            # ---- phase A: xbown cast + per-group ald ----
            with (
                tc.tile_pool(name="pa_x", bufs=1) as pax,
                tc.tile_pool(name="pa_psD", bufs=2, space="PSUM") as papD,
            ):
                xall = pax.tile([F, NPC], f32)
                nc.sync.dma_start(xall[:], xTown[:])
                nc.scalar.activation(xbown_sb[:, 0:NPC], xall[:], AF.Copy)
                if NPAD > NPC:
                    nc.gpsimd.memset(xbown_sb[:, NPC:NPAD], 0.0)
                for t in range(NT):
                    nt_ = min(128, NPC - 128 * t)
                    xt = xall[:, 128 * t : 128 * t + nt_]
                    psald = papD.tile([128, H1], f32)
                    nc.tensor.matmul(
                        out=psald[:nt_, :], lhsT=xt, rhs=adf_sb[:], start=True, stop=True
                    )
                    if nt_ < 128:
                        nc.gpsimd.memset(aldg_sb[:, H1 * t : H1 * (t + 1)], 0.0)
                    nc.vector.tensor_copy(
                        aldg_sb[:nt_, H1 * t : H1 * (t + 1)], psald[:nt_, :]
                    )

            qctr = [0]  # rotating SWDGE queue assignment across gather issues
            sctr = [0]  # xstage ring slot counter

            def issue_src_batch(pool, table_pair, p, b, bufs, tagp, esz, transpose,
                                rawpool=None):
                """Gather one src batch on the next queue (idxs preloaded).

                transpose=True lands the batch as a matmul-ready lhsT
                [F, CB*128] via a DRAM staging hop (gather -> DRAM ->
                transposed XBAR load)."""
                nb = [NBL, NBH][p]
                assert b < nb
                it = [sxl_sb, sxh_sb][p]
                xraw = (rawpool or pool).tile([128, CB, esz], bf16, tag=f"xr{tagp}{p}")
                table = table_pair[p]
                nsl = [sched["NSL"], sched["NSH"]][p]
                nval = min(CB, nsl - b * CB) * 128
                q = qctr[0] % NQ
                qctr[0] += 1
                nc.gpsimd.dma_gather(
                    out_ap=xraw[:],
                    in_ap=table,
                    idxs_ap=it[:, b * CB * 8 : (b + 1) * CB * 8],
                    num_idxs=CB * 128,
                    num_idxs_reg=nval,
                    elem_size=esz,
                    transpose=False,
                    single_packet=False,
                    queue_num=q,
                )
                if transpose:
                    slot = sctr[0] % GBUFS
                    sctr[0] += 1
                    stview = xstage[slot].rearrange("(a e) f -> e a f", a=CB)
                    nc.sync.dma_start(stview, xraw[:])
                    xT = pool.tile([128, CB * 128], bf16, tag=f"xT{tagp}{p}")
                    nc.sync.dma_start(xT[:], xstage[slot][:], transpose=True)
                    bufs[(p, b)] = xT
                else:
                    bufs[(p, b)] = xraw

            def issue_dst_batch(ipool, spool, stpool, Sbuf, STbuf, b):
                """Build S (scatter) and ST (expand) matrices for a d-batch."""
                dc = ipool.tile([128, DCW], bf16, tag="dc")
                nc.sync.dma_start(dc[:], dcomb[:, b * DCW : (b + 1) * DCW])
                S = spool.tile([128, CB * 128], bf16, tag="S")
                nc.vector.tensor_tensor(
                    out=S[:].rearrange("p (a n) -> p a n", a=CB),
                    in0=dc[:, 0:CB].to_broadcast([128, CB, 128]),
                    in1=iotab_sb[:]
                    .rearrange("p (a n) -> p a n", a=1)
                    .broadcast_to([128, CB, 128]),
                    op=OP.is_equal,
                )
                Sbuf[b] = S
                ST = stpool.tile([128, CB * 128], bf16, tag="ST")
                nc.vector.tensor_tensor(
                    out=ST[:],
                    in0=iotap_sb[:].to_broadcast([128, CB * 128]),
                    in1=dc[:, CB:DCW],
                    op=OP.is_equal,
                )
                STbuf[b] = ST

            def xT_sl(ch, xbufs):
                """lhsT [F=128, 128 edges] slice for a chunk (gathered or own)."""
                if ch["sslot"] is None:
                    g = ch["g"]
                    return xbown_sb[:, 128 * g : 128 * (g + 1)]
                p, ss = ch["p"], ch["sslot"]
                return xbufs[(p, ss // CB)][:, (ss % CB) * 128 : (ss % CB) * 128 + 128]

            # ---- phase B: layer-1 edge processing ----
            if "B" in PHASES:
                with (
                    tc.tile_pool(name="gx", bufs=GBUFS) as gxp,
                    tc.tile_pool(name="gxr", bufs=4) as gxr,
                    tc.tile_pool(name="gi", bufs=2) as gip,
                    tc.tile_pool(name="sS", bufs=2) as ssp,
                    tc.tile_pool(name="sT", bufs=2) as stp,
                    tc.tile_pool(name="sE", bufs=2) as sep,
                    tc.tile_pool(name="msg", bufs=3) as msp,
                    tc.tile_pool(name="fin", bufs=2) as fip,
                    tc.tile_pool(name="psH", bufs=2, space="PSUM") as psH,
                    tc.tile_pool(name="psN", bufs=2, space="PSUM") as psN,
                    tc.tile_pool(name="psE", bufs=2, space="PSUM") as psE,
                    tc.tile_pool(name="psD", bufs=2, space="PSUM") as psD,
                ):
                    xbufs = {}  # (p, batch) -> tile
                    ebanks = {}
                    Sbuf = {}
                    STbuf = {}
                    psums = {}  # g -> (psumN, psumD)

                    def finalize_group(gg, psumN, psumD):
                        dd = fip.tile([128, H1], f32, tag="dd")
                        nc.vector.tensor_scalar_add(dd[:], psumD[:], 1e-16)
                        rc = fip.tile([128, H1], f32, tag="rc")
                        nc.vector.reciprocal(rc[:], dd[:])
                        o1 = fip.tile([128, HC], f32, tag="o1")
                        nc.vector.tensor_tensor(
                            out=o1[:].rearrange("p (h k) -> p h k", h=H1),
                            in0=psumN[:].rearrange("p (h k) -> p h k", h=H1),
                            in1=rc[:]
                            .rearrange("p (h o) -> p h o", o=1)
                            .broadcast_to([128, H1, HID]),
                            op=OP.mult,
                        )
                        # elu = min(exp(x)-1, relu(x)); relu on the scalar engine
                        expo = fip.tile([128, HC], f32, tag="expo")
                        nc.scalar.activation(expo[:], o1[:], AF.Exp)
                        rel = fip.tile([128, HC], f32, tag="rel")
                        nc.scalar.activation(rel[:], o1[:], AF.Relu)
                        h2t = fip.tile([128, HC], bf16, tag="h2t")
                        nc.vector.scalar_tensor_tensor(
                            out=h2t[:],
                            in0=expo[:],
                            scalar=-1.0,
                            in1=rel[:],
                            op0=OP.add,
                            op1=OP.min,
                        )
                        nc.sync.dma_start(
                            h2_dram[128 * gg : 128 * (gg + 1), :], h2t[:]
                        )

                    for ch in chunks:
                        c, g, p, ss = ch["c"], ch["g"], ch["p"], ch["sslot"]
                        bd, jd = c // CB, c % CB
                        if ss is not None and (p, ss // CB) not in xbufs:
                            issue_src_batch(
                                gxp,
                                (xb_full[0:SPLIT, :], xb_full[SPLIT:N, :]),
                                p,
                                ss // CB,
                                xbufs,
                                "1",
                                F,
                                True,
                                rawpool=gxr,
                            )
                        if bd not in Sbuf:
                            issue_dst_batch(gip, ssp, stp, Sbuf, STbuf, bd)
                        if bd not in ebanks:
                            # full psum bank to keep zero regions private
                            ebanks[bd] = psE.tile([128, 512], f32, tag="E", name=f"E{bd}")
                        # e = als + ald accumulated in PSUM: two matmuls per chunk
                        nc.tensor.matmul(
                            out=ebanks[bd][:, jd * 8 : jd * 8 + 8],
                            lhsT=xT_sl(ch, xbufs),
                            rhs=asb_sb[:],
                            start=(jd == 0),
                            stop=False,
                        )
                        nc.tensor.matmul(
                            out=ebanks[bd][:, jd * 8 : jd * 8 + 8],
                            lhsT=STbuf[bd][:, jd * 128 : jd * 128 + 128],
                            rhs=aldg_sb[:, H1 * g : H1 * (g + 1)],
                            start=False,
                            stop=(jd == CB - 1 or c == NCH - 1),
                        )

                        # once we hit the last chunk of a d-batch, run exp + messages
                        if jd == CB - 1 or c == NCH - 1:
                            nchb = (c % CB) + 1  # chunks in this batch
                            if os.environ.get("GAT_EXPSB", "0") == "1":
                                ebk = sep.tile([128, CB * H1], f32, tag="ebk")
                                nc.vector.tensor_copy(
                                    ebk[:, 0 : nchb * 8], ebanks[bd][:, 0 : nchb * 8]
                                )
                            else:
                                ebk = ebanks[bd]
                            # exp(leaky_relu(x)) == max(exp(SLOPE*x), exp(x))
                            ex1 = sep.tile([128, CB * H1], bf16, tag="ex1")
                            nc.scalar.activation(
                                ex1[:, 0 : nchb * 8], ebk[:, 0 : nchb * 8], AF.Exp,
                                scale=SLOPE,
                            )
                            ex2 = sep.tile([128, CB * H1], bf16, tag="ex2")
                            nc.scalar.activation(
                                ex2[:, 0 : nchb * 8], ebk[:, 0 : nchb * 8], AF.Exp
                            )
                            ex = sep.tile([128, CB, H1], bf16, tag="ex")
                            nc.vector.tensor_tensor(
                                out=ex[:, 0:nchb, :].rearrange("p a n -> p (a n)"),
                                in0=ex2[:, 0 : nchb * 8],
                                in1=ex1[:, 0 : nchb * 8],
                                op=OP.max,
                            )
                            # per-chunk: h recompute, weighted message, accumulate.
                            # N/D matmuls are deferred one chunk so the PE can run
                            # h(c+1) while the vector engine builds mt(c).
                            pend = None

                            def emit_ND(cc2, mt2):
                                ch2 = chunks[cc2]
                                if ch2["first"]:
                                    psums[ch2["g"]] = (
                                        psN.tile([128, HC], f32, tag="N", name=f"N{ch2['g']}"),
                                        psD.tile([128, H1], f32, tag="D", name=f"D{ch2['g']}"),
                                    )
                                psumN, psumD = psums[ch2["g"]]
                                Ssl = Sbuf[bd][:, (cc2 % CB) * 128 : (cc2 % CB) * 128 + 128]
                                nc.tensor.matmul(
                                    out=psumN[:],
                                    lhsT=Ssl,
                                    rhs=mt2[:],
                                    start=ch2["first"],
                                    stop=ch2["last"],
                                )
                                nc.tensor.matmul(
                                    out=psumD[:],
                                    lhsT=Ssl,
                                    rhs=ex[:, cc2 % CB, :],
                                    start=ch2["first"],
                                    stop=ch2["last"],
                                )
                                if ch2["last"]:
                                    finalize_group(ch2["g"], psumN, psumD)
                                    del psums[ch2["g"]]

                            for cc in range(bd * CB, min((bd + 1) * CB, NCH)):
                                psh = psH.tile([128, HC], f32, tag="H")
                                nc.tensor.matmul(
                                    out=psh[:],
                                    lhsT=xT_sl(chunks[cc], xbufs),
                                    rhs=w1b_sb[:],
                                    start=True,
                                    stop=True,
                                )
                                mt = msp.tile([128, HC], bf16, tag="m")
                                nc.vector.tensor_tensor(
                                    out=mt[:].rearrange("p (h k) -> p h k", h=H1),
                                    in0=psh[:].rearrange("p (h k) -> p h k", h=H1),
                                    in1=ex[:, cc % CB, :]
                                    .rearrange("p (h o) -> p h o", o=1)
                                    .broadcast_to([128, H1, HID]),
                                    op=OP.mult,
                                )
                                if pend is not None:
                                    emit_ND(*pend)
                                pend = (cc, mt)
                            emit_ND(*pend)

            # ---- phase C: g2 table ([1 | g2 | als2] rows) + AllGather ----
            if "C" in PHASES:
                with (
                    tc.tile_pool(name="pc_s", bufs=3) as pc,
                    tc.tile_pool(name="pc_h2t", bufs=1) as ph2,
                    tc.tile_pool(name="pc_ps", bufs=2, space="PSUM") as pcp,
                ):
                    w2_sb = pc.tile([128, 4, HID + 2], bf16, tag="w2")
                    nc.sync.dma_start(
                        w2_sb[:], w2e[:].rearrange("(i p) c -> p i c", i=4)
                    )
                    h2T = ph2.tile([128, 4, NPAD], bf16)
                    for i in range(4):
                        nc.sync.dma_start(
                            h2T[:, i, :],
                            h2_dram[:, 128 * i : 128 * (i + 1)],
                            transpose=True,
                        )
                    for t in range(NT):
                        nt_ = min(128, NPC - 128 * t)
                        ps = pcp.tile([128, HID + 2], f32)
                        for i in range(4):
                            nc.tensor.matmul(
                                out=ps[:],
                                lhsT=h2T[:, i, 128 * t : 128 * t + 128],
                                rhs=w2_sb[:, i, :],
                                start=(i == 0),
                                stop=(i == 3),
                            )
                        gv = pc.tile([128, G2W], bf16, tag="gv")
                        nc.vector.tensor_copy(gv[:, 0:1], ones_col[:])
                        nc.vector.tensor_copy(gv[:nt_, 1 : HID + 2], ps[:nt_, 0 : HID + 1])
                        if nt_ < 128:
                            nc.gpsimd.memset(ald2g_sb[:, t : t + 1], 0.0)
                        nc.vector.tensor_copy(
                            ald2g_sb[:nt_, t : t + 1], ps[:nt_, HID + 1 : HID + 2]
                        )
                        if nt_ < 128:
                            # zero pad rows so self-chunk reads stay finite
                            nc.gpsimd.memset(g2own_sb[:, G2W * t : G2W * (t + 1)], 0.0)
                            nc.vector.tensor_copy(
                                g2own_sb[:nt_, G2W * t : G2W * (t + 1)], gv[:nt_, :]
                            )
                        else:
                            nc.vector.tensor_copy(
                                g2own_sb[:, G2W * t : G2W * (t + 1)], gv[:]
                            )
                        nc.sync.dma_start(
                            g2_own[128 * t : 128 * t + nt_, 0:G2W], gv[:nt_, :]
                        )
                    nc.gpsimd.collective_compute(
                        "AllGather",
                        mybir.AluOpType.bypass,
                        replica_groups=RG,
                        ins=[g2_own[:]],
                        outs=[g2_full[:]],
                    )

            # ---- phase D: layer-2 edge processing + pooling ----
            if "D" in PHASES:
                with (
                    tc.tile_pool(name="g2x", bufs=GBUFS) as g2xp,
                    tc.tile_pool(name="gi2", bufs=2) as gip2,
                    tc.tile_pool(name="sS2", bufs=2) as ssp2,
                    tc.tile_pool(name="sT2", bufs=2) as stp2,
                    tc.tile_pool(name="sE2", bufs=2) as sep2,
                    tc.tile_pool(name="m2", bufs=2) as msp2,
                    tc.tile_pool(name="fin2", bufs=2) as fip2,
                    tc.tile_pool(name="psN2", bufs=2, space="PSUM") as psN2,
                    tc.tile_pool(name="psE2", bufs=2, space="PSUM") as psE2,
                    tc.tile_pool(name="psP", bufs=1, space="PSUM") as psP,
                ):
                    xbufs2 = {}
                    Sbuf2 = {}
                    STbuf2 = {}
                    e2banks = {}
                    psums2 = {}
                    psumPool = psP.tile([HID + 1, G], f32)

                    for ch in chunks:
                        c, g, p, ss = ch["c"], ch["g"], ch["p"], ch["sslot"]
                        bd, jd = c // CB, c % CB
                        if ss is not None and (p, ss // CB) not in xbufs2:
                            issue_src_batch(
                                g2xp,
                                (g2_full[0:SPLIT, :], g2_full[SPLIT:N, :]),
                                p,
                                ss // CB,
                                xbufs2,
                                "2",
                                128,
                                False,
                            )
                        if bd not in Sbuf2:
                            issue_dst_batch(gip2, ssp2, stp2, Sbuf2, STbuf2, bd)
                        if bd not in e2banks:
                            e2banks[bd] = psE2.tile(
                                [128, 512], f32, tag="E2", name=f"E2_{bd}"
                            )
                        # ald2[dst] per edge via transposed selection matrix
                        nc.tensor.matmul(
                            out=e2banks[bd][:, jd : jd + 1],
                            lhsT=STbuf2[bd][:, jd * 128 : jd * 128 + 128],
                            rhs=ald2g_sb[:, g : g + 1],
                            start=(jd == 0),
                            stop=(jd == CB - 1 or c == NCH - 1),
                        )

                        if jd == CB - 1 or c == NCH - 1:
                            nchb = (c % CB) + 1
                            # er2 = als2[src] + ald2 (psum); gathered or own col
                            er = sep2.tile([128, CB], f32, tag="er")
                            for (c0, r, rp, s0) in runs:
                                if c0 // CB != bd:
                                    continue
                                nc.vector.tensor_tensor(
                                    out=er[:, c0 % CB : c0 % CB + r].rearrange(
                                        "p (a o) -> p a o", o=1
                                    ),
                                    in0=xbufs2[(rp, s0 // CB)][
                                        :, s0 % CB : s0 % CB + r, HID + 1 : HID + 2
                                    ],
                                    in1=e2banks[bd][:, c0 % CB : c0 % CB + r].rearrange(
                                        "p (a o) -> p a o", o=1
                                    ),
                                    op=OP.add,
                                )
                            for cc in range(bd * CB, min((bd + 1) * CB, NCH)):
                                ch2 = chunks[cc]
                                if ch2["sslot"] is not None:
                                    continue
                                gg = ch2["g"]
                                nc.vector.tensor_tensor(
                                    out=er[:, cc % CB : cc % CB + 1],
                                    in0=g2own_sb[
                                        :, G2W * gg + HID + 1 : G2W * gg + HID + 2
                                    ],
                                    in1=e2banks[bd][:, cc % CB : cc % CB + 1],
                                    op=OP.add,
                                )
                            ex1 = sep2.tile([128, CB], bf16, tag="ex1")
                            nc.scalar.activation(
                                ex1[:, 0:nchb], er[:, 0:nchb], AF.Exp, scale=SLOPE
                            )
                            ex2 = sep2.tile([128, CB], bf16, tag="ex2")
                            nc.scalar.activation(ex2[:, 0:nchb], er[:, 0:nchb], AF.Exp)
                            ex = sep2.tile([128, CB], bf16, tag="ex")
                            nc.vector.tensor_tensor(
                                out=ex[:, 0:nchb],
                                in0=ex2[:, 0:nchb],
                                in1=ex1[:, 0:nchb],
                                op=OP.max,
                            )
                            # weighted messages [ex | g2*ex] (den col rides along)
                            me = msp2.tile([128, CB, HID + 1], bf16, tag="me")
                            for (c0, r, rp, s0) in runs:
                                if c0 // CB != bd:
                                    continue
                                nc.vector.tensor_tensor(
                                    out=me[:, c0 % CB : c0 % CB + r, :],
                                    in0=xbufs2[(rp, s0 // CB)][
                                        :, s0 % CB : s0 % CB + r, 0 : HID + 1
                                    ],
                                    in1=ex[:, c0 % CB : c0 % CB + r]
                                    .rearrange("p (a o) -> p a o", o=1)
                                    .broadcast_to([128, r, HID + 1]),
                                    op=OP.mult,
                                )
                            for cc in range(bd * CB, min((bd + 1) * CB, NCH)):
                                ch2 = chunks[cc]
                                if ch2["sslot"] is not None:
                                    continue
                                gg = ch2["g"]
                                nc.vector.tensor_tensor(
                                    out=me[:, cc % CB, :],
                                    in0=g2own_sb[:, G2W * gg : G2W * gg + HID + 1],
                                    in1=ex[:, cc % CB : cc % CB + 1].to_broadcast(
                                        [128, HID + 1]
                                    ),
                                    op=OP.mult,
                                )
                            for cc in range(bd * CB, min((bd + 1) * CB, NCH)):
                                ch2 = chunks[cc]
                                if ch2["first"]:
                                    psums2[ch2["g"]] = psN2.tile(
                                        [128, HID + 1], f32, tag="N2", name=f"N2_{ch2['g']}"
                                    )
                                psumN2 = psums2[ch2["g"]]
                                Ssl = Sbuf2[bd][:, (cc % CB) * 128 : (cc % CB) * 128 + 128]
                                nc.tensor.matmul(
                                    out=psumN2[:],
                                    lhsT=Ssl,
                                    rhs=me[:, cc % CB, :],
                                    start=ch2["first"],
                                    stop=ch2["last"],
                                )
                                if ch2["last"]:
                                    gg = ch2["g"]
                                    dd = fip2.tile([128, 1], f32, tag="dd")
                                    nc.vector.tensor_scalar_add(
                                        dd[:], psumN2[:, 0:1], 1e-16
                                    )
                                    rc = fip2.tile([128, 1], f32, tag="rc")
                                    nc.vector.reciprocal(rc[:], dd[:])
                                    o2e = fip2.tile([128, HID + 1], bf16, tag="o2e")
                                    nc.vector.tensor_scalar(
                                        out=o2e[:, 1 : HID + 1],
                                        in0=psumN2[:, 1 : HID + 1],
                                        scalar1=rc[:],
                                        scalar2=None,
                                        op0=OP.mult,
                                    )
                                    nc.vector.tensor_copy(o2e[:, 0:1], ones_col[:])
                                    nc.tensor.matmul(
                                        out=psumPool[:],
                                        lhsT=o2e[:],
                                        rhs=gt_sb[:, gg * G : (gg + 1) * G],
                                        start=(gg == 0),
                                        stop=(gg == NT - 1),
                                    )

                    # pool -> DRAM -> AllReduce
                    plsb = fip2.tile([HID + 1, G], f32, tag="pl")
                    nc.vector.tensor_copy(plsb[:], psumPool[:])
                    nc.sync.dma_start(pool_own[:], plsb[:])
                    nc.gpsimd.collective_compute(
                        "AllReduce",
                        mybir.AluOpType.add,
                        replica_groups=RG,
                        ins=[pool_own[:]],
                        outs=[pool_ar[:]],
                    )

            # ---- phase E: fc + log_softmax (replicated) ----
            # pool rows: [count | sums(HID)]; fcwb rows: [fc_b row | fc_w]
            if "E" in PHASES:
                with (
                    tc.tile_pool(name="pe_s", bufs=1) as pe,
                    tc.tile_pool(name="pe_ps", bufs=1, space="PSUM") as pep,
                ):
                    nc.sync.dma_start(pool_loc[:], pool_ar[:])
                    poolA = pe.tile([HID + 1, G], f32)
                    nc.sync.dma_start(poolA[:], pool_loc[:])
                    fcw_sb = pe.tile([HID + 1, CLS], f32)
                    nc.sync.dma_start(fcw_sb[:], fcwb[:])
                    cnt = pe.tile([G, 1], f32)
                    nc.sync.dma_start(cnt[:], pool_loc[0:1, :].rearrange("a g -> g a"))
                    lg_ps = pep.tile([G, CLS], f32)
                    nc.tensor.matmul(
                        out=lg_ps[:], lhsT=poolA[:], rhs=fcw_sb[:], start=True, stop=True
                    )
                    cnt1 = pe.tile([G, 1], f32)
                    nc.vector.tensor_scalar_max(cnt1[:], cnt[:], 1.0)
                    rcnt = pe.tile([G, 1], f32)
                    nc.vector.reciprocal(rcnt[:], cnt1[:])
                    lg = pe.tile([G, CLS], f32)
                    nc.vector.tensor_scalar(
                        out=lg[:], in0=lg_ps[:], scalar1=rcnt[:], scalar2=None, op0=OP.mult
                    )
                    mx = pe.tile([G, 1], f32)
                    nc.vector.reduce_max(mx[:], lg[:], axis=mybir.AxisListType.X)
                    lgs = pe.tile([G, CLS], f32)
                    nc.vector.tensor_scalar(
                        out=lgs[:], in0=lg[:], scalar1=mx[:], scalar2=None, op0=OP.subtract
                    )
                    ex = pe.tile([G, CLS], f32)
                    sume = pe.tile([G, 1], f32)
                    nc.scalar.activation(ex[:], lgs[:], AF.Exp, accum_out=sume[:])
                    lse = pe.tile([G, 1], f32)
                    nc.scalar.activation(lse[:], sume[:], AF.Ln)
                    res = pe.tile([G, CLS], f32)
                    nc.vector.tensor_scalar(
                        out=res[:], in0=lgs[:], scalar1=lse[:], scalar2=None, op0=OP.subtract
                    )
                    nc.sync.dma_start(out[:], res[:])

    nc.compile()
    return nc


def make_inputs(x, edge_index, batch, W1, a_src1, a_dst1, b1, W2, a_src2, a_dst2, b2, fc_w, fc_b):
    """Host-side preprocessing -> (sched, in_maps)."""
    x = np.asarray(x, np.float32)
    edge_index = np.asarray(edge_index, np.int64)
    batch = np.asarray(batch, np.int64)
    W1 = np.asarray(W1, np.float32)
    a_src1 = np.asarray(a_src1, np.float32)
    a_dst1 = np.asarray(a_dst1, np.float32)
    W2 = np.asarray(W2, np.float32)
    a_src2 = np.asarray(a_src2, np.float32)
    a_dst2 = np.asarray(a_dst2, np.float32)
    fc_w = np.asarray(fc_w, np.float32)
    fc_b = np.asarray(fc_b, np.float32)
    b1 = np.asarray(b1, np.float32)
    b2 = np.asarray(b2, np.float32)
    assert not np.any(b1), "kernel assumes b1 == 0 (setup_inputs gives zeros)"

    sched, per_core = preprocess(edge_index, batch)

    W1r = W1.reshape(F, H1, HID)
    A_s = np.einsum("fhc,hc->fh", W1r, a_src1).astype(np.float32)
    A_d = np.einsum("fhc,hc->fh", W1r, a_dst1).astype(np.float32)
    w_as2 = (W2 @ a_src2[0]).astype(np.float32)
    w_ad2 = (W2 @ a_dst2[0]).astype(np.float32)
    w2e = np.concatenate([W2, w_as2[:, None], w_ad2[:, None]], axis=1)
    fc_b2 = fc_b + b2 @ fc_w
    # pool rows are [count | sums]: bias row FIRST
    fcwb = np.concatenate([fc_b2[None, :], fc_w], axis=0).astype(np.float32)

    common = dict(
        xb_full=x.astype(BF16),
        w1b=W1.astype(BF16),
        asb=A_s.astype(BF16),
        adf=A_d,
        w2e=w2e.astype(BF16),
        fcwb=fcwb,
    )
    in_maps = []
    for k in range(NCORES):
        pc = per_core[k]
        m = dict(common)
        m["xTown"] = np.ascontiguousarray(x[NPC * k : NPC * (k + 1)].T)
        m["sidx_lo"] = pc["sidx_lo"]
        m["sidx_hi"] = pc["sidx_hi"]
        # combined per-batch dst info: CB dstloc columns (S build) followed by
        # CB*128 partition-replicated row values (ST build)
        dt16 = pc["dstlocT"].astype(BF16)
        nbd = dt16.shape[1] // CB
        dcw = CB + CB * 128
        dcomb = np.empty((128, nbd * dcw), BF16)
        for b in range(nbd):
            dcomb[:, b * dcw : b * dcw + CB] = dt16[:, b * CB : (b + 1) * CB]
            dcomb[:, b * dcw + CB : (b + 1) * dcw] = np.broadcast_to(
                dt16[:, b * CB : (b + 1) * CB].T.reshape(1, -1), (128, CB * 128)
            )
        m["dcomb"] = dcomb
        m["gtT"] = pc["gtT"]
        in_maps.append(m)
    return sched, in_maps


def kernel(**inputs):
    sched, in_maps = make_inputs(**inputs)
    nc = build_program(sched)
    from concourse.bass_utils import run_bass_kernel_spmd

    trace = bool(int(os.environ.get("GAT_TRACE", "0")))
    res = run_bass_kernel_spmd(
        nc, in_maps, core_ids=list(range(NCORES)), trace=trace
    )
    if trace and res.exec_time_ns is not None:
        print(f"HW exec time: {res.exec_time_ns} ns")
        kernel.last_exec_time_ns = res.exec_time_ns
    return np.asarray(res.results[0]["out"], np.float32)
